# revision 39
# baseline (speedup 1.0000x reference)
"""Trainium2 Bass kernel for nn_DecoderBlock (dense_transformer).

Sharding (8 NeuronCores): core c handles batch b = c//4 and head-group
r = c%4 (3 of 12 heads).  Attention is tensor-parallel over heads within
each 4-core batch group.  Per-512-row chunk, a 4-core AllGather (bf16,
192KB per rank) collects all 768 head-features; each core selects its
owned 128-row strip with per-core 0/1 scalars (keeping the SPMD program
rank-symmetric), out-projects it with the full wo, adds the residual,
and LayerNorms.  The FFN runs sequence-parallel on the owned 512 rows
with full FFN weights (prefetched during attention), so no further
communication is needed.

All matmul operands are bf16 (same PE rate as fp32r at these tile
shapes, half the DMA/SBUF); accumulation stays fp32 in PSUM.  The
attention inner loop interleaves the three heads round-by-round
(score x3 -> exp x3 -> attnV x3) so the PE never stalls on the Scalar
engine's exp and the HAM clock gate stays warm.  Softmax needs no
max-subtraction (scores are bounded for this input distribution); the
denominator comes free from a ones-column appended to V.
"""

import numpy as np
import ml_dtypes

import concourse.bass as bass
import concourse.tile as tile
import concourse.mybir as mybir
from concourse import bacc
from concourse.bass_utils import run_bass_kernel_spmd

# Model dims (hardcoded per the problem spec).
B = 2
S = 2048
D = 768
H = 12
DK = 64
DFF = 3072
EPS = 1e-5

NCORES = 8
RANKS = 4                  # cores per batch group
HPC = H // RANKS           # heads per core = 3
HD = HPC * DK              # head features per core = 192
ROWS = S // RANKS          # owned rows per core = 512
P = 128
NBLK = S // P              # 16 key blocks per batch
KO_D = D // P              # 6 feature chunks of d_model
KO_F = DFF // P            # 24 feature chunks of d_ff
QC = S // 512              # 4 query chunks of 512

F32 = mybir.dt.float32
BF16 = mybir.dt.bfloat16
BF = ml_dtypes.bfloat16

_CACHE = {}


def _build():
    from contextlib import ExitStack

    nc = bacc.Bacc(None, target_bir_lowering=False)

    # ---- external I/O ----
    xbT = nc.dram_tensor("xbT", [D, S], BF16, kind="ExternalInput")
    xownbo = nc.dram_tensor("xownbo", [ROWS, D], F32, kind="ExternalInput")
    # q/k weights padded into 4 chunks of 128: [q0 q1 | q2 pad | k0 k1 | k2 pad]
    # so each head's q and k slices sit at matching partition offsets.
    wqk = nc.dram_tensor("wqk", [D, 4 * P], BF16, kind="ExternalInput")
    bqk = nc.dram_tensor("bqk", [4 * P], F32, kind="ExternalInput")
    wvp = nc.dram_tensor("wvp", [D, HD], BF16, kind="ExternalInput")
    bvb = nc.dram_tensor("bvb", [P, HD], BF16, kind="ExternalInput")
    wo = nc.dram_tensor("wo", [D, D], BF16, kind="ExternalInput")
    sel = nc.dram_tensor("sel", [P, RANKS], F32, kind="ExternalInput")
    w1 = nc.dram_tensor("w1", [D, DFF], BF16, kind="ExternalInput")
    b1 = nc.dram_tensor("b1", [DFF], F32, kind="ExternalInput")
    w2 = nc.dram_tensor("w2", [DFF, D], BF16, kind="ExternalInput")
    b2b = nc.dram_tensor("b2b", [P, D], F32, kind="ExternalInput")
    g1b = nc.dram_tensor("g1b", [P, D], F32, kind="ExternalInput")
    be1b = nc.dram_tensor("be1b", [P, D], F32, kind="ExternalInput")
    g2b = nc.dram_tensor("g2b", [P, D], F32, kind="ExternalInput")
    be2b = nc.dram_tensor("be2b", [P, D], F32, kind="ExternalInput")
    ident_in = nc.dram_tensor("ident", [P, P], F32, kind="ExternalInput")
    masks_in = nc.dram_tensor("masks", [P, P], BF16, kind="ExternalInput")
    out = nc.dram_tensor("out", [ROWS, D], F32, kind="ExternalOutput")

    groups = [[0, 1, 2, 3], [4, 5, 6, 7]]

    with tile.TileContext(nc) as tc, ExitStack() as outer:
        consts = outer.enter_context(tc.tile_pool(name="consts", bufs=1))
        lnsmall = outer.enter_context(tc.tile_pool(name="lnsmall", bufs=2))
        dram = outer.enter_context(tc.tile_pool(name="dram", bufs=1, space="DRAM"))

        # persistent working set
        qkpool = outer.enter_context(tc.tile_pool(name="qkpool", bufs=1))
        expp = outer.enter_context(tc.tile_pool(name="expp", bufs=4))
        attnp = outer.enter_context(tc.tile_pool(name="attnp", bufs=2))
        pbp = outer.enter_context(tc.tile_pool(name="pbp", bufs=2))
        atall = outer.enter_context(tc.tile_pool(name="atall", bufs=2))
        atsel = outer.enter_context(tc.tile_pool(name="atsel", bufs=1))
        wop = outer.enter_context(tc.tile_pool(name="wop", bufs=1))
        xop = outer.enter_context(tc.tile_pool(name="xop", bufs=2))
        ffnbuf = outer.enter_context(tc.tile_pool(name="ffnbuf", bufs=1))
        w1p = outer.enter_context(tc.tile_pool(name="w1p", bufs=1))

        # ---- ACT table warmup + projection inputs first (critical path) ----
        scratch = consts.tile([1, 8], F32)
        nc.vector.memset(scratch[:], 0.25)
        for fn in (mybir.ActivationFunctionType.Exp,
                   mybir.ActivationFunctionType.Identity,
                   mybir.ActivationFunctionType.Relu,
                   mybir.ActivationFunctionType.Sqrt,
                   mybir.ActivationFunctionType.Ln):
            nc.scalar.activation(scratch[:], scratch[:], fn)
        bqk_sb = consts.tile([P, 4], F32)
        nc.sync.dma_start(bqk_sb[:], bqk.rearrange("(mo p) -> p mo", p=P))

        # ===== frame 1: xT + projection weights (freed before w2 loads) =====
        fr1 = ExitStack()
        wqkv = fr1.enter_context(tc.tile_pool(name="wqkv", bufs=1))
        xtpool = fr1.enter_context(tc.tile_pool(name="xtpool", bufs=1))

        wqk_sb = wqkv.tile([P, KO_D, 4 * P], BF16)
        wvp_sb = wqkv.tile([P, KO_D, HD], BF16)
        xT = xtpool.tile([P, KO_D, S], BF16)
        for ko in range(KO_D):
            nc.sync.dma_start(
                wqk_sb[:, ko, :], wqk[ko * P:(ko + 1) * P, :])
            nc.sync.dma_start(xT[:, ko, :], xbT[ko * P:(ko + 1) * P, :])
        for ko in range(KO_D):
            nc.sync.dma_start(
                wvp_sb[:, ko, :], wvp[ko * P:(ko + 1) * P, :])

        # ---- remaining constants (not needed until later phases) ----
        ident = consts.tile([P, P], F32)
        nc.sync.dma_start(ident[:], ident_in[:])
        # lower-triangle keep-mask; identical for every diagonal block
        mask_sb = consts.tile([P, P], BF16)
        nc.sync.dma_start(mask_sb[:], masks_in[:])
        b1_sb = consts.tile([P, KO_F], F32)
        nc.sync.dma_start(b1_sb[:], b1.rearrange("(mo p) -> p mo", p=P))
        bvb_sb = consts.tile([P, HD], BF16)
        nc.sync.dma_start(bvb_sb[:], bvb[:])
        b2b_sb = consts.tile([P, D], F32)
        nc.sync.dma_start(b2b_sb[:], b2b[:])
        g1_sb = consts.tile([P, D], F32)
        nc.sync.dma_start(g1_sb[:], g1b[:])
        be1_sb = consts.tile([P, D], F32)
        nc.sync.dma_start(be1_sb[:], be1b[:])
        g2_sb = consts.tile([P, D], F32)
        nc.sync.dma_start(g2_sb[:], g2b[:])
        be2_sb = consts.tile([P, D], F32)
        nc.sync.dma_start(be2_sb[:], be2b[:])
        eps_sb = consts.tile([P, 1], F32)
        nc.vector.memset(eps_sb[:], EPS)
        ones_sb = consts.tile([1, DK], BF16)
        nc.vector.memset(ones_sb[:], 1.0)

        wo_sb = wop.tile([P, KO_D, D], BF16)
        for ko in range(KO_D):
            nc.sync.dma_start(wo_sb[:, ko, :], wo[ko * P:(ko + 1) * P, :])
        sel_sb = consts.tile([P, RANKS], F32)
        nc.sync.dma_start(sel_sb[:], sel[:])

        # ---- q/k projection (feature-major) ----
        prj = ExitStack()
        prjps = prj.enter_context(
            tc.tile_pool(name="prjps", bufs=2, space="PSUM"))
        prjpv = prj.enter_context(
            tc.tile_pool(name="prjpv", bufs=2, space="PSUM"))

        # chunk layout: 0=[q0 q1], 1=[q2 pad], 2=[k0 k1], 3=[k2 pad]
        qk_sb = qkpool.tile([P, 4, S], BF16)
        for mo in range(4):
            for nq in range(QC):
                ps = prjps.tile([P, 512], F32, tag="pp")
                for ko in range(KO_D):
                    nc.tensor.matmul(
                        ps[:],
                        wqk_sb[:, ko, mo * P:(mo + 1) * P],
                        xT[:, ko, nq * 512:(nq + 1) * 512],
                        start=(ko == 0),
                        stop=(ko == KO_D - 1),
                    )
                nc.scalar.activation(
                    qk_sb[:, mo, nq * 512:(nq + 1) * 512],
                    ps[:],
                    mybir.ActivationFunctionType.Identity,
                    bias=bqk_sb[:, mo:mo + 1],
                )

        # ---- v projection (row-major, per-head with ones column) ----
        v_sb = qkpool.tile([P, NBLK, HPC, DK + 1], BF16)
        nc.vector.memset(v_sb[:, :, :, DK], 1.0)
        for blk in range(NBLK):
            psv = prjpv.tile([P, HD], F32, tag="pv")
            for ko in range(KO_D):
                nc.tensor.matmul(
                    psv[:],
                    xT[:, ko, blk * P:(blk + 1) * P],
                    wvp_sb[:, ko, :],
                    start=(ko == 0),
                    stop=(ko == KO_D - 1),
                )
            nc.vector.tensor_add(
                v_sb[:, blk, :, 0:DK],
                psv[:].rearrange("p (h d) -> p h d", h=HPC),
                bvb_sb[:].rearrange("p (h d) -> p h d", h=HPC),
            )
        prj.close()
        fr1.close()  # xT / projection weights no longer needed

        # ===== frame 2: w2 weights (reuse xT's space) =====
        fr2 = ExitStack()
        w2p = fr2.enter_context(tc.tile_pool(name="w2p", bufs=1))
        w1_sb = w1p.tile([P, KO_D, DFF], BF16)
        w2_sb = w2p.tile([P, KO_F, D], BF16)

        # deferred FFN-weight prefetch: one chunk per attention round
        prefetch = []
        for ko in range(KO_D):
            prefetch.append(
                (w1_sb[:, ko, :], w1[ko * P:(ko + 1) * P, :]))
        for ko in range(KO_F):
            prefetch.append(
                (w2_sb[:, ko, :], w2[ko * P:(ko + 1) * P, :]))

        def pop_prefetch():
            if prefetch:
                dst, src = prefetch.pop(0)
                nc.sync.dma_start(dst, src)

        # ===== attention: interleaved heads, chunk-major (descending) =====
        apsum = ExitStack()
        pssp = apsum.enter_context(
            tc.tile_pool(name="pssp", bufs=3, space="PSUM"))
        pop = apsum.enter_context(
            tc.tile_pool(name="pop", bufs=3, space="PSUM"))
        # shared 2-bank ring for all serial accumulation chains
        # (softmax broadcast, out-proj, transposes, FFN1, FFN2)
        chn = apsum.enter_context(
            tc.tile_pool(name="chn", bufs=2, space="PSUM"))

        ag_in = [dram.tile([HD, 512], BF16, name=f"ag_in{c}")
                 for c in range(QC)]
        ag_out = [dram.tile([RANKS * HD, 512], BF16, name=f"ag_out{c}")
                  for c in range(QC)]

        u_sb = ffnbuf.tile([P, QC, D], F32)
        uT = ffnbuf.tile([P, KO_D, ROWS], BF16)
        rdenp = lnsmall  # reuse small pool for rden tiles

        def rounds_chunk(c):
            """scores/exp/attnV for chunk c, 3 heads interleaved.
            Diagonal key blocks skip the fully-masked query prefix."""
            nkb = 4 * c + 4
            po = [pop.tile([DK + 1, 512], F32, tag="po", name=f"po{c}_{h}")
                  for h in range(HPC)]
            for kb in range(nkb):
                pop_prefetch()
                j = kb - 4 * c
                w0 = max(j, 0) * P  # fully-masked query prefix
                pss = []
                for h in range(HPC):
                    q_mo, q_off = h // 2, (h % 2) * DK
                    k_mo, k_off = 2 + h // 2, (h % 2) * DK
                    p = pssp.tile([P, 512], F32, tag="pss")
                    nc.tensor.matmul(
                        p[:, w0:512],
                        qk_sb[k_off:k_off + DK, k_mo, kb * P:(kb + 1) * P],
                        qk_sb[q_off:q_off + DK, q_mo,
                              c * 512 + w0:(c + 1) * 512],
                        start=True,
                        stop=True,
                    )
                    pss.append(p)
                exs = []
                for h in range(HPC):
                    ex = expp.tile([P, 512], BF16, tag="ex")
                    if w0 > 0:
                        nc.vector.memset(ex[:, 0:w0], 0.0)
                    nc.scalar.activation(
                        ex[:, w0:512], pss[h][:, w0:512],
                        mybir.ActivationFunctionType.Exp,
                        scale=float(1.0 / np.sqrt(DK)),
                    )
                    if j >= 0:
                        # triangular boundary block only
                        nc.vector.tensor_mul(ex[:, w0:w0 + P],
                                             ex[:, w0:w0 + P],
                                             mask_sb[:])
                    exs.append(ex)
                for h in range(HPC):
                    nc.tensor.matmul(
                        po[h][:],
                        v_sb[:, kb, h, :],
                        exs[h][:],
                        start=(kb == 0),
                        stop=(kb == nkb - 1),
                    )
            return po

        def fin_chunk(c, po):
            """softmax-normalize chunk c and kick off its AllGather.
            1/denominator comes from exp(-ln(den)) on the Scalar engine —
            DVE reciprocal on a [1, 512] tile is pathologically serial."""
            attn = attnp.tile([P, 2, 512], BF16, tag="attn", name=f"attn{c}")
            for h in range(HPC):
                lnden = rdenp.tile([1, 512], F32, tag="lnden")
                nc.scalar.activation(lnden[:], po[h][DK:DK + 1, :],
                                     mybir.ActivationFunctionType.Ln)
                dinv = rdenp.tile([1, 512], BF16, tag="dinv")
                nc.scalar.activation(dinv[:], lnden[:],
                                     mybir.ActivationFunctionType.Exp,
                                     scale=-1.0)
                # broadcast 1/den to 64 partitions via PE outer product
                pb = chn.tile([DK, 512], F32, tag="chn", name=f"pb{c}_{h}")
                nc.tensor.matmul(pb[:], ones_sb[:], dinv[:],
                                 start=True, stop=True)
                pb_sb = pbp.tile([DK, 512], BF16, tag="pbsb")
                nc.scalar.copy(pb_sb[:], pb[:])
                a_mo, a_off = (h * DK) // P, (h * DK) % P
                nc.vector.tensor_mul(
                    attn[a_off:a_off + DK, a_mo, :],
                    po[h][0:DK, :],
                    pb_sb[:],
                )
            nc.sync.dma_start(ag_in[c][0:P, :], attn[:, 0, :])
            nc.sync.dma_start(ag_in[c][P:HD, :], attn[0:HD - P, 1, :])
            nc.gpsimd.collective_compute(
                "AllGather",
                mybir.AluOpType.bypass,
                replica_groups=groups,
                ins=[ag_in[c][:]],
                outs=[ag_out[c][:]],
            )
            xo = xop.tile([P, D], F32, tag="xo", name=f"xo{c}")
            nc.sync.dma_start(xo[:], xownbo[c * P:(c + 1) * P, :])
            return xo

        def load_chunk(c):
            att_full = atall.tile([P, KO_D, 512], BF16, tag="atfull",
                                  name=f"atfull{c}")
            nc.gpsimd.dma_start(
                att_full[:], ag_out[c].rearrange("(ko p) s -> p ko s", p=P))
            return att_full

        def outproj_chunk(c, att_full, xo):
            """select the owned strip, out-project it, add residual, LN1,
            and transpose into uT."""
            # per-core 0/1 selection of the owned 128-row strip
            att_s = atsel.tile([P, KO_D, P], BF16, tag="atsel",
                               name=f"atsel{c}")
            tmp = atsel.tile([P, KO_D, P], BF16, tag="atsel_t")
            nc.vector.tensor_scalar(
                att_s[:], att_full[:, :, 0:P],
                scalar1=sel_sb[:, 0:1], scalar2=None,
                op0=mybir.AluOpType.mult)
            for jr in range(1, RANKS):
                nc.vector.tensor_scalar(
                    tmp[:], att_full[:, :, jr * P:(jr + 1) * P],
                    scalar1=sel_sb[:, jr:jr + 1], scalar2=None,
                    op0=mybir.AluOpType.mult)
                nc.vector.tensor_add(att_s[:], att_s[:], tmp[:])
            psy = [chn.tile([P, 384], F32, tag="chn", name=f"psy{c}_{no}")
                   for no in range(2)]
            for ko in range(KO_D):
                for no in range(2):
                    nc.tensor.matmul(
                        psy[no][:],
                        att_s[:, ko, :],
                        wo_sb[:, ko, no * 384:(no + 1) * 384],
                        start=(ko == 0),
                        stop=(ko == KO_D - 1),
                    )
            for no in range(2):
                sl = slice(no * 384, (no + 1) * 384)
                nc.vector.tensor_add(u_sb[:, c, sl], psy[no][:], xo[:, sl])
            _layernorm(nc, lnsmall, u_sb[:, c, :], eps_sb, g1_sb, be1_sb)
            for fo in range(KO_D):
                pst = chn.tile([P, P], F32, tag="chn", name=f"tp{c}_{fo}")
                nc.tensor.transpose(
                    pst[:], u_sb[:, c, fo * P:(fo + 1) * P], ident[:])
                nc.vector.tensor_copy(uT[:, fo, c * P:(c + 1) * P], pst[:])

        h_sb = ffnbuf.tile([P, KO_F, ROWS], BF16)
        stage3 = fr2.enter_context(tc.tile_pool(name="stage3", bufs=2))

        def ffn1_pass(lo, hi):
            """h[:, :, lo:hi] = relu(uT[:, :, lo:hi] @ w1 + b1)."""
            for mo in range(KO_F):
                psh = chn.tile([P, hi - lo], F32, tag="chn",
                               name=f"psh{mo}_{lo}")
                for ko in range(KO_D):
                    nc.tensor.matmul(
                        psh[:],
                        w1_sb[:, ko, mo * P:(mo + 1) * P],
                        uT[:, ko, lo:hi],
                        start=(ko == 0),
                        stop=(ko == KO_D - 1),
                    )
                nc.scalar.activation(
                    h_sb[:, mo, lo:hi], psh[:],
                    mybir.ActivationFunctionType.Relu,
                    bias=b1_sb[:, mo:mo + 1],
                )

        def ffn2_blk(blk):
            """FFN2 + residual + LN2 + store for one owned strip."""
            psf = [chn.tile([P, 384], F32, tag="chn", name=f"psf{blk}_{no}")
                   for no in range(2)]
            for ko in range(KO_F):
                for no in range(2):
                    nc.tensor.matmul(
                        psf[no][:],
                        h_sb[:, ko, blk * P:(blk + 1) * P],
                        w2_sb[:, ko, no * 384:(no + 1) * 384],
                        start=(ko == 0),
                        stop=(ko == KO_F - 1),
                    )
            ost = stage3.tile([P, D], F32, tag="ost")
            for no in range(2):
                sl = slice(no * 384, (no + 1) * 384)
                nc.vector.tensor_add(ost[:, sl], u_sb[:, blk, sl], psf[no][:])
            nc.vector.tensor_add(ost[:], ost[:], b2b_sb[:])
            _layernorm(nc, lnsmall, ost[:], eps_sb, g2_sb, be2_sb)
            nc.sync.dma_start(out[blk * P:(blk + 1) * P, :], ost[:])

        # rounds back-to-back; exactly one collective in flight at a time
        # (multiple pending collectives make the ncfw round-robin crawl),
        # each att_full load emitted right after its AG's trigger.
        # chunk 0 first: its AllGather starts ~70us earlier, so the serial
        # collective chain drains while the big chunks' rounds still run;
        # out-projections slot in as gathers land.
        xos, atts = {}, {}
        po0 = rounds_chunk(0); xos[0] = fin_chunk(0, po0)
        po3 = rounds_chunk(3); atts[0] = load_chunk(0); xos[3] = fin_chunk(3, po3)
        po2 = rounds_chunk(2); atts[3] = load_chunk(3); xos[2] = fin_chunk(2, po2)
        outproj_chunk(0, atts.pop(0), xos.pop(0))
        po1 = rounds_chunk(1); atts[2] = load_chunk(2); xos[1] = fin_chunk(1, po1)
        outproj_chunk(3, atts.pop(3), xos.pop(3))
        atts[1] = load_chunk(1)
        outproj_chunk(2, atts.pop(2), xos.pop(2))
        outproj_chunk(1, atts.pop(1), xos.pop(1))
        ffn1_pass(0, ROWS)
        ffn2_blk(3)
        ffn2_blk(2)
        ffn2_blk(1)
        ffn2_blk(0)

        apsum.close()
        fr2.close()

    nc.compile()
    return nc


def _layernorm(nc, pool, z, eps_sb, g_sb, b_sb):
    """In-place LayerNorm over the free dim (768) of z [128, 768]."""
    sub = 256
    nsub = D // sub
    stats = pool.tile([P, nsub, nc.vector.BN_STATS_DIM], F32, tag="ln_stats")
    mv = pool.tile([P, nc.vector.BN_AGGR_DIM], F32, tag="ln_mv")
    zr = z.rearrange("p (n s) -> p n s", s=sub)
    for sg in range(nsub):
        nc.vector.bn_stats(stats[:, sg, :], zr[:, sg, :])
    nc.vector.bn_aggr(mv[:], stats[:])
    std = pool.tile([P, 1], F32, tag="ln_std")
    nc.scalar.activation(
        std[:], mv[:, 1:2], mybir.ActivationFunctionType.Sqrt, bias=eps_sb[:]
    )
    rstd = pool.tile([P, 1], F32, tag="ln_rstd")
    nc.vector.reciprocal(rstd[:], std[:])
    nc.vector.tensor_scalar(
        z, z,
        scalar1=mv[:, 0:1],
        scalar2=rstd[:],
        op0=mybir.AluOpType.subtract,
        op1=mybir.AluOpType.mult,
    )
    nc.vector.tensor_mul(z, z, g_sb[:])
    nc.vector.tensor_add(z, z, b_sb[:])


def _host_inputs(x, wq, bq, wk, bk, wv, bv, wo, bo, w1, b1, w2, b2,
                 g1, be1, g2, be2):
    """Build the per-core input maps."""
    f = np.float32
    ident = np.eye(P, dtype=f)
    # triangle keep-mask for diagonal blocks: keep iff key p <= query q
    masks = (np.arange(P)[:, None] <= np.arange(P)[None, :]).astype(BF)

    xT = [np.ascontiguousarray(x[b].T).astype(BF) for b in range(B)]

    shared = {
        "wo": np.ascontiguousarray(wo).astype(BF),
        "w1": np.ascontiguousarray(w1).astype(BF),
        "b1": np.ascontiguousarray(b1, dtype=f),
        "w2": np.ascontiguousarray(w2).astype(BF),
        "b2b": np.broadcast_to(b2, (P, D)).astype(f),
        "g1b": np.broadcast_to(g1, (P, D)).astype(f),
        "be1b": np.broadcast_to(be1, (P, D)).astype(f),
        "g2b": np.broadcast_to(g2, (P, D)).astype(f),
        "be2b": np.broadcast_to(be2, (P, D)).astype(f),
        "ident": ident,
        "masks": masks,
    }

    in_maps = []
    for c in range(NCORES):
        b, r = divmod(c, RANKS)
        hs = slice(r * HD, (r + 1) * HD)
        # [q0 q1 | q2 pad | k0 k1 | k2 pad]
        wqkp = np.zeros((D, 4 * P), f)
        wqkp[:, 0:P] = wq[:, hs][:, 0:P]
        wqkp[:, P:P + DK] = wq[:, hs][:, P:HD]
        wqkp[:, 2 * P:3 * P] = wk[:, hs][:, 0:P]
        wqkp[:, 3 * P:3 * P + DK] = wk[:, hs][:, P:HD]
        bqkp = np.zeros(4 * P, f)
        bqkp[0:P] = bq[hs][0:P]
        bqkp[P:P + DK] = bq[hs][P:HD]
        bqkp[2 * P:3 * P] = bk[hs][0:P]
        bqkp[3 * P:3 * P + DK] = bk[hs][P:HD]
        # owned rows: strip r of each 512-chunk, with bo folded in
        xown = np.concatenate(
            [x[b, cc * 512 + r * P: cc * 512 + (r + 1) * P] for cc in range(QC)],
            axis=0,
        ) + bo[None, :]
        selm = np.zeros((P, RANKS), f)
        selm[:, r] = 1.0
        m = {
            "xbT": xT[b],
            "xownbo": np.ascontiguousarray(xown, dtype=f),
            "wqk": wqkp.astype(BF),
            "bqk": bqkp,
            "wvp": np.ascontiguousarray(wv[:, hs]).astype(BF),
            "bvb": np.broadcast_to(bv[hs], (P, HD)).astype(BF),
            "sel": selm,
        }
        m.update(shared)
        in_maps.append({k: np.ascontiguousarray(v) for k, v in m.items()})
    return in_maps


def _get_nc():
    if "nc" not in _CACHE:
        _CACHE["nc"] = _build()
    return _CACHE["nc"]


def run(inputs, **kw):
    """Run on hardware; returns (output, BassKernelResults)."""
    nc = _get_nc()
    in_maps = _host_inputs(**inputs)
    res = run_bass_kernel_spmd(nc, in_maps, core_ids=list(range(NCORES)), **kw)
    out = np.empty((B, S, D), np.float32)
    for core in range(NCORES):
        b, r = divmod(core, RANKS)
        o = res.results[core]["out"]
        for c in range(QC):
            out[b, c * 512 + r * P: c * 512 + (r + 1) * P, :] = o[c * P:(c + 1) * P]
    return out, res


def kernel(**inputs):
    return run(inputs)[0]


# revision 40
# speedup vs baseline: 1.1541x; 1.1541x over previous
"""Trainium2 Bass kernel for nn_DecoderBlock (dense_transformer).

Sharding (8 NeuronCores): core c handles batch b = c//4 and head-group
r = c%4 (3 of 12 heads).  Attention is tensor-parallel over heads within
each 4-core batch group.  Per-512-row chunk, a 4-core AllGather (bf16,
192KB per rank) collects all 768 head-features; each core selects its
owned 128-row strip with per-core 0/1 scalars (keeping the SPMD program
rank-symmetric), out-projects it with the full wo, adds the residual,
and LayerNorms.  The FFN runs sequence-parallel on the owned 512 rows
with full FFN weights (prefetched during attention), so no further
communication is needed.

All matmul operands are bf16 (same PE rate as fp32r at these tile
shapes, half the DMA/SBUF); accumulation stays fp32 in PSUM.  The
attention inner loop interleaves the three heads round-by-round
(score x3 -> exp x3 -> attnV x3) so the PE never stalls on the Scalar
engine's exp and the HAM clock gate stays warm.  Softmax needs no
max-subtraction (scores are bounded for this input distribution); the
denominator comes free from a ones-column appended to V.
"""

import numpy as np
import ml_dtypes

import concourse.bass as bass
import concourse.tile as tile
import concourse.mybir as mybir
from concourse import bacc
from concourse.bass_utils import run_bass_kernel_spmd

# Model dims (hardcoded per the problem spec).
B = 2
S = 2048
D = 768
H = 12
DK = 64
DFF = 3072
EPS = 1e-5

NCORES = 8
RANKS = 4                  # cores per batch group
HPC = H // RANKS           # heads per core = 3
HD = HPC * DK              # head features per core = 192
ROWS = S // RANKS          # owned rows per core = 512
P = 128
NBLK = S // P              # 16 key blocks per batch
KO_D = D // P              # 6 feature chunks of d_model
KO_F = DFF // P            # 24 feature chunks of d_ff
QC = S // 512              # 4 query chunks of 512

F32 = mybir.dt.float32
BF16 = mybir.dt.bfloat16
BF = ml_dtypes.bfloat16

_CACHE = {}


def _build():
    from contextlib import ExitStack

    nc = bacc.Bacc(None, target_bir_lowering=False)

    # ---- external I/O ----
    xbT = nc.dram_tensor("xbT", [D, S], BF16, kind="ExternalInput")
    xownbo = nc.dram_tensor("xownbo", [ROWS, D], F32, kind="ExternalInput")
    # q/k weights padded into 4 chunks of 128: [q0 q1 | q2 pad | k0 k1 | k2 pad]
    # so each head's q and k slices sit at matching partition offsets.
    wqk = nc.dram_tensor("wqk", [D, 4 * P], BF16, kind="ExternalInput")
    bqk = nc.dram_tensor("bqk", [4 * P], F32, kind="ExternalInput")
    wvp = nc.dram_tensor("wvp", [D, HD], BF16, kind="ExternalInput")
    bvb = nc.dram_tensor("bvb", [P, HD], BF16, kind="ExternalInput")
    wo = nc.dram_tensor("wo", [D, D], BF16, kind="ExternalInput")
    sel = nc.dram_tensor("sel", [P, RANKS], F32, kind="ExternalInput")
    w1 = nc.dram_tensor("w1", [D, DFF], BF16, kind="ExternalInput")
    b1 = nc.dram_tensor("b1", [DFF], F32, kind="ExternalInput")
    w2 = nc.dram_tensor("w2", [DFF, D], BF16, kind="ExternalInput")
    b2b = nc.dram_tensor("b2b", [P, D], F32, kind="ExternalInput")
    g1b = nc.dram_tensor("g1b", [P, D], F32, kind="ExternalInput")
    be1b = nc.dram_tensor("be1b", [P, D], F32, kind="ExternalInput")
    g2b = nc.dram_tensor("g2b", [P, D], F32, kind="ExternalInput")
    be2b = nc.dram_tensor("be2b", [P, D], F32, kind="ExternalInput")
    ident_in = nc.dram_tensor("ident", [P, P], F32, kind="ExternalInput")
    masks_in = nc.dram_tensor("masks", [P, P], BF16, kind="ExternalInput")
    out = nc.dram_tensor("out", [ROWS, D], F32, kind="ExternalOutput")

    groups = [[0, 1, 2, 3], [4, 5, 6, 7]]

    with tile.TileContext(nc) as tc, ExitStack() as outer:
        consts = outer.enter_context(tc.tile_pool(name="consts", bufs=1))
        lnsmall = outer.enter_context(tc.tile_pool(name="lnsmall", bufs=2))
        dram = outer.enter_context(tc.tile_pool(name="dram", bufs=1, space="DRAM"))

        # persistent working set
        qkpool = outer.enter_context(tc.tile_pool(name="qkpool", bufs=1))
        expp = outer.enter_context(tc.tile_pool(name="expp", bufs=4))
        attnp = outer.enter_context(tc.tile_pool(name="attnp", bufs=2))
        pbp = outer.enter_context(tc.tile_pool(name="pbp", bufs=2))
        atall = outer.enter_context(tc.tile_pool(name="atall", bufs=2))
        atsel = outer.enter_context(tc.tile_pool(name="atsel", bufs=1))
        wop = outer.enter_context(tc.tile_pool(name="wop", bufs=1))
        xop = outer.enter_context(tc.tile_pool(name="xop", bufs=2))
        ffnbuf = outer.enter_context(tc.tile_pool(name="ffnbuf", bufs=1))
        w1p = outer.enter_context(tc.tile_pool(name="w1p", bufs=1))

        # ---- ACT table warmup + projection inputs first (critical path) ----
        scratch = consts.tile([1, 8], F32)
        nc.vector.memset(scratch[:], 0.25)
        for fn in (mybir.ActivationFunctionType.Exp,
                   mybir.ActivationFunctionType.Identity,
                   mybir.ActivationFunctionType.Relu,
                   mybir.ActivationFunctionType.Sqrt,
                   mybir.ActivationFunctionType.Ln):
            nc.scalar.activation(scratch[:], scratch[:], fn)
        bqk_sb = consts.tile([P, 4], F32)
        nc.sync.dma_start(bqk_sb[:], bqk.rearrange("(mo p) -> p mo", p=P))

        # ===== frame 1: xT + projection weights (freed before w2 loads) =====
        fr1 = ExitStack()
        wqkv = fr1.enter_context(tc.tile_pool(name="wqkv", bufs=1))
        xtpool = fr1.enter_context(tc.tile_pool(name="xtpool", bufs=1))

        wqk_sb = wqkv.tile([P, KO_D, 4 * P], BF16)
        wvp_sb = wqkv.tile([P, KO_D, HD], BF16)
        xT = xtpool.tile([P, KO_D, S], BF16)
        for ko in range(KO_D):
            nc.sync.dma_start(
                wqk_sb[:, ko, :], wqk[ko * P:(ko + 1) * P, :])
            nc.sync.dma_start(xT[:, ko, :], xbT[ko * P:(ko + 1) * P, :])
        for ko in range(KO_D):
            nc.sync.dma_start(
                wvp_sb[:, ko, :], wvp[ko * P:(ko + 1) * P, :])

        # ---- remaining constants (not needed until later phases) ----
        ident = consts.tile([P, P], F32)
        nc.sync.dma_start(ident[:], ident_in[:])
        # lower-triangle keep-mask; identical for every diagonal block
        mask_sb = consts.tile([P, P], BF16)
        nc.sync.dma_start(mask_sb[:], masks_in[:])
        b1_sb = consts.tile([P, KO_F], F32)
        nc.sync.dma_start(b1_sb[:], b1.rearrange("(mo p) -> p mo", p=P))
        bvb_sb = consts.tile([P, HD], BF16)
        nc.sync.dma_start(bvb_sb[:], bvb[:])
        b2b_sb = consts.tile([P, D], F32)
        nc.sync.dma_start(b2b_sb[:], b2b[:])
        g1_sb = consts.tile([P, D], F32)
        nc.sync.dma_start(g1_sb[:], g1b[:])
        be1_sb = consts.tile([P, D], F32)
        nc.sync.dma_start(be1_sb[:], be1b[:])
        g2_sb = consts.tile([P, D], F32)
        nc.sync.dma_start(g2_sb[:], g2b[:])
        be2_sb = consts.tile([P, D], F32)
        nc.sync.dma_start(be2_sb[:], be2b[:])
        eps_sb = consts.tile([P, 1], F32)
        nc.vector.memset(eps_sb[:], EPS)
        ones_sb = consts.tile([1, DK], BF16)
        nc.vector.memset(ones_sb[:], 1.0)

        wo_sb = wop.tile([P, KO_D, D], BF16)
        for ko in range(KO_D):
            nc.sync.dma_start(wo_sb[:, ko, :], wo[ko * P:(ko + 1) * P, :])
        sel_sb = consts.tile([P, RANKS], F32)
        nc.sync.dma_start(sel_sb[:], sel[:])

        # ---- q/k projection (feature-major) ----
        prj = ExitStack()
        prjps = prj.enter_context(
            tc.tile_pool(name="prjps", bufs=2, space="PSUM"))
        prjpv = prj.enter_context(
            tc.tile_pool(name="prjpv", bufs=2, space="PSUM"))

        # chunk layout: 0=[q0 q1], 1=[q2 pad], 2=[k0 k1], 3=[k2 pad]
        qk_sb = qkpool.tile([P, 4, S], BF16)
        for mo in range(4):
            for nq in range(QC):
                ps = prjps.tile([P, 512], F32, tag="pp")
                for ko in range(KO_D):
                    nc.tensor.matmul(
                        ps[:],
                        wqk_sb[:, ko, mo * P:(mo + 1) * P],
                        xT[:, ko, nq * 512:(nq + 1) * 512],
                        start=(ko == 0),
                        stop=(ko == KO_D - 1),
                    )
                nc.scalar.activation(
                    qk_sb[:, mo, nq * 512:(nq + 1) * 512],
                    ps[:],
                    mybir.ActivationFunctionType.Identity,
                    bias=bqk_sb[:, mo:mo + 1],
                )

        # ---- v projection (row-major, per-head with ones column) ----
        v_sb = qkpool.tile([P, NBLK, HPC, DK + 1], BF16)
        nc.vector.memset(v_sb[:, :, :, DK], 1.0)
        for blk in range(NBLK):
            psv = prjpv.tile([P, HD], F32, tag="pv")
            for ko in range(KO_D):
                nc.tensor.matmul(
                    psv[:],
                    xT[:, ko, blk * P:(blk + 1) * P],
                    wvp_sb[:, ko, :],
                    start=(ko == 0),
                    stop=(ko == KO_D - 1),
                )
            nc.vector.tensor_add(
                v_sb[:, blk, :, 0:DK],
                psv[:].rearrange("p (h d) -> p h d", h=HPC),
                bvb_sb[:].rearrange("p (h d) -> p h d", h=HPC),
            )
        prj.close()
        fr1.close()  # xT / projection weights no longer needed

        # ===== frame 2: w2 weights (reuse xT's space) =====
        fr2 = ExitStack()
        w2p = fr2.enter_context(tc.tile_pool(name="w2p", bufs=1))
        w1_sb = w1p.tile([P, KO_D, DFF], BF16)
        w2_sb = w2p.tile([P, KO_F, D], BF16)

        # deferred FFN-weight prefetch: one chunk per attention round
        prefetch = []
        for ko in range(KO_D):
            prefetch.append(
                (w1_sb[:, ko, :], w1[ko * P:(ko + 1) * P, :]))
        for ko in range(KO_F):
            prefetch.append(
                (w2_sb[:, ko, :], w2[ko * P:(ko + 1) * P, :]))

        def pop_prefetch():
            if prefetch:
                dst, src = prefetch.pop(0)
                nc.sync.dma_start(dst, src)

        # ===== attention: interleaved heads, chunk-major (descending) =====
        apsum = ExitStack()
        pssp = apsum.enter_context(
            tc.tile_pool(name="pssp", bufs=3, space="PSUM"))
        pop = apsum.enter_context(
            tc.tile_pool(name="pop", bufs=3, space="PSUM"))
        # shared 2-bank ring for all serial accumulation chains
        # (softmax broadcast, out-proj, transposes, FFN1, FFN2)
        chn = apsum.enter_context(
            tc.tile_pool(name="chn", bufs=2, space="PSUM"))

        ag_in = [dram.tile([HD, 512], BF16, name=f"ag_in{c}")
                 for c in range(QC)]
        ag_out = [dram.tile([RANKS * HD, 512], BF16, name=f"ag_out{c}")
                  for c in range(QC)]

        u_sb = ffnbuf.tile([P, QC, D], F32)
        uT = ffnbuf.tile([P, KO_D, ROWS], BF16)
        rdenp = lnsmall  # reuse small pool for rden tiles

        def rounds_chunk(c):
            """scores/exp/attnV for chunk c, 3 heads interleaved.
            Diagonal key blocks skip the fully-masked query prefix."""
            nkb = 4 * c + 4
            po = [pop.tile([DK + 1, 512], F32, tag="po", name=f"po{c}_{h}")
                  for h in range(HPC)]
            for kb in range(nkb):
                pop_prefetch()
                j = kb - 4 * c
                w0 = max(j, 0) * P  # fully-masked query prefix
                pss = []
                for h in range(HPC):
                    q_mo, q_off = h // 2, (h % 2) * DK
                    k_mo, k_off = 2 + h // 2, (h % 2) * DK
                    p = pssp.tile([P, 512], F32, tag="pss")
                    nc.tensor.matmul(
                        p[:, w0:512],
                        qk_sb[k_off:k_off + DK, k_mo, kb * P:(kb + 1) * P],
                        qk_sb[q_off:q_off + DK, q_mo,
                              c * 512 + w0:(c + 1) * 512],
                        start=True,
                        stop=True,
                    )
                    pss.append(p)
                exs = []
                for h in range(HPC):
                    ex = expp.tile([P, 512], BF16, tag="ex")
                    if w0 > 0:
                        nc.vector.memset(ex[:, 0:w0], 0.0)
                    nc.scalar.activation(
                        ex[:, w0:512], pss[h][:, w0:512],
                        mybir.ActivationFunctionType.Exp,
                        scale=float(1.0 / np.sqrt(DK)),
                    )
                    if j >= 0:
                        # triangular boundary block only
                        nc.vector.tensor_mul(ex[:, w0:w0 + P],
                                             ex[:, w0:w0 + P],
                                             mask_sb[:])
                    exs.append(ex)
                for h in range(HPC):
                    nc.tensor.matmul(
                        po[h][:],
                        v_sb[:, kb, h, :],
                        exs[h][:],
                        start=(kb == 0),
                        stop=(kb == nkb - 1),
                    )
            return po

        def fin_chunk(c, po):
            """softmax-normalize chunk c and kick off its AllGather.
            1/denominator comes from exp(-ln(den)) on the Scalar engine —
            DVE reciprocal on a [1, 512] tile is pathologically serial."""
            attn = attnp.tile([P, 2, 512], BF16, tag="attn", name=f"attn{c}")
            for h in range(HPC):
                lnden = rdenp.tile([1, 512], F32, tag="lnden")
                nc.scalar.activation(lnden[:], po[h][DK:DK + 1, :],
                                     mybir.ActivationFunctionType.Ln)
                dinv = rdenp.tile([1, 512], BF16, tag="dinv")
                nc.scalar.activation(dinv[:], lnden[:],
                                     mybir.ActivationFunctionType.Exp,
                                     scale=-1.0)
                # broadcast 1/den to 64 partitions via PE outer product
                pb = chn.tile([DK, 512], F32, tag="chn", name=f"pb{c}_{h}")
                nc.tensor.matmul(pb[:], ones_sb[:], dinv[:],
                                 start=True, stop=True)
                pb_sb = pbp.tile([DK, 512], BF16, tag="pbsb")
                nc.scalar.copy(pb_sb[:], pb[:])
                a_mo, a_off = (h * DK) // P, (h * DK) % P
                nc.vector.tensor_mul(
                    attn[a_off:a_off + DK, a_mo, :],
                    po[h][0:DK, :],
                    pb_sb[:],
                )
            nc.sync.dma_start(ag_in[c][0:P, :], attn[:, 0, :])
            nc.sync.dma_start(ag_in[c][P:HD, :], attn[0:HD - P, 1, :])
            nc.gpsimd.collective_compute(
                "AllGather",
                mybir.AluOpType.bypass,
                replica_groups=groups,
                ins=[ag_in[c][:]],
                outs=[ag_out[c][:]],
            )
            xo = xop.tile([P, D], F32, tag="xo", name=f"xo{c}")
            nc.sync.dma_start(xo[:], xownbo[c * P:(c + 1) * P, :])
            return xo

        def load_chunk(c):
            att_full = atall.tile([P, KO_D, 512], BF16, tag="atfull",
                                  name=f"atfull{c}")
            nc.gpsimd.dma_start(
                att_full[:], ag_out[c].rearrange("(ko p) s -> p ko s", p=P))
            return att_full

        def outproj_chunk(c, att_full, xo):
            """select the owned strip, out-project it, add residual, LN1,
            and transpose into uT."""
            # per-core 0/1 selection of the owned 128-row strip
            att_s = atsel.tile([P, KO_D, P], BF16, tag="atsel",
                               name=f"atsel{c}")
            tmp = atsel.tile([P, KO_D, P], BF16, tag="atsel_t")
            nc.vector.tensor_scalar(
                att_s[:], att_full[:, :, 0:P],
                scalar1=sel_sb[:, 0:1], scalar2=None,
                op0=mybir.AluOpType.mult)
            for jr in range(1, RANKS):
                nc.vector.tensor_scalar(
                    tmp[:], att_full[:, :, jr * P:(jr + 1) * P],
                    scalar1=sel_sb[:, jr:jr + 1], scalar2=None,
                    op0=mybir.AluOpType.mult)
                nc.vector.tensor_add(att_s[:], att_s[:], tmp[:])
            psy = [chn.tile([P, 384], F32, tag="chn", name=f"psy{c}_{no}")
                   for no in range(2)]
            for ko in range(KO_D):
                for no in range(2):
                    nc.tensor.matmul(
                        psy[no][:],
                        att_s[:, ko, :],
                        wo_sb[:, ko, no * 384:(no + 1) * 384],
                        start=(ko == 0),
                        stop=(ko == KO_D - 1),
                    )
            for no in range(2):
                sl = slice(no * 384, (no + 1) * 384)
                nc.vector.tensor_add(u_sb[:, c, sl], psy[no][:], xo[:, sl])
            _layernorm(nc, lnsmall, u_sb[:, c, :], eps_sb, g1_sb, be1_sb)
            for fo in range(KO_D):
                pst = chn.tile([P, P], F32, tag="chn", name=f"tp{c}_{fo}")
                nc.tensor.transpose(
                    pst[:], u_sb[:, c, fo * P:(fo + 1) * P], ident[:])
                nc.vector.tensor_copy(uT[:, fo, c * P:(c + 1) * P], pst[:])

        h_sb = ffnbuf.tile([P, KO_F, ROWS], BF16)
        stage3 = fr2.enter_context(tc.tile_pool(name="stage3", bufs=2))

        def ffn1_pass(lo, hi):
            """h[:, :, lo:hi] = relu(uT[:, :, lo:hi] @ w1 + b1)."""
            for mo in range(KO_F):
                psh = chn.tile([P, hi - lo], F32, tag="chn",
                               name=f"psh{mo}_{lo}")
                for ko in range(KO_D):
                    nc.tensor.matmul(
                        psh[:],
                        w1_sb[:, ko, mo * P:(mo + 1) * P],
                        uT[:, ko, lo:hi],
                        start=(ko == 0),
                        stop=(ko == KO_D - 1),
                    )
                nc.scalar.activation(
                    h_sb[:, mo, lo:hi], psh[:],
                    mybir.ActivationFunctionType.Relu,
                    bias=b1_sb[:, mo:mo + 1],
                )

        def ffn2_blk(blk):
            """FFN2 + residual + LN2 + store for one owned strip."""
            psf = [chn.tile([P, 384], F32, tag="chn", name=f"psf{blk}_{no}")
                   for no in range(2)]
            for ko in range(KO_F):
                for no in range(2):
                    nc.tensor.matmul(
                        psf[no][:],
                        h_sb[:, ko, blk * P:(blk + 1) * P],
                        w2_sb[:, ko, no * 384:(no + 1) * 384],
                        start=(ko == 0),
                        stop=(ko == KO_F - 1),
                    )
            ost = stage3.tile([P, D], F32, tag="ost")
            for no in range(2):
                sl = slice(no * 384, (no + 1) * 384)
                nc.vector.tensor_add(ost[:, sl], u_sb[:, blk, sl], psf[no][:])
            nc.vector.tensor_add(ost[:], ost[:], b2b_sb[:])
            _layernorm(nc, lnsmall, ost[:], eps_sb, g2_sb, be2_sb)
            nc.sync.dma_start(out[blk * P:(blk + 1) * P, :], ost[:])

        # rounds back-to-back; exactly one collective in flight at a time
        # (multiple pending collectives make the ncfw round-robin crawl),
        # each att_full load emitted right after its AG's trigger.
        # rounds back-to-back; exactly one collective in flight at a time
        # (multiple pending collectives make the ncfw round-robin crawl),
        # each att_full load emitted right after its AG's trigger.
        xos, atts = {}, {}
        po3 = rounds_chunk(3); xos[3] = fin_chunk(3, po3)
        po2 = rounds_chunk(2); atts[3] = load_chunk(3); xos[2] = fin_chunk(2, po2)
        po1 = rounds_chunk(1); atts[2] = load_chunk(2); xos[1] = fin_chunk(1, po1)
        po0 = rounds_chunk(0); atts[1] = load_chunk(1); xos[0] = fin_chunk(0, po0)
        atts[0] = load_chunk(0)
        # drain: out-projections as AllGathers land, FFN filling the gaps
        outproj_chunk(3, atts.pop(3), xos.pop(3))
        ffn1_pass(3 * P, 4 * P)
        outproj_chunk(2, atts.pop(2), xos.pop(2))
        ffn1_pass(2 * P, 3 * P)
        ffn2_blk(3)
        outproj_chunk(1, atts.pop(1), xos.pop(1))
        ffn2_blk(2)
        outproj_chunk(0, atts.pop(0), xos.pop(0))
        ffn1_pass(0, 2 * P)
        ffn2_blk(1)
        ffn2_blk(0)

        apsum.close()
        fr2.close()

    nc.compile()
    return nc


def _layernorm(nc, pool, z, eps_sb, g_sb, b_sb):
    """In-place LayerNorm over the free dim (768) of z [128, 768]."""
    sub = 256
    nsub = D // sub
    stats = pool.tile([P, nsub, nc.vector.BN_STATS_DIM], F32, tag="ln_stats")
    mv = pool.tile([P, nc.vector.BN_AGGR_DIM], F32, tag="ln_mv")
    zr = z.rearrange("p (n s) -> p n s", s=sub)
    for sg in range(nsub):
        nc.vector.bn_stats(stats[:, sg, :], zr[:, sg, :])
    nc.vector.bn_aggr(mv[:], stats[:])
    std = pool.tile([P, 1], F32, tag="ln_std")
    nc.scalar.activation(
        std[:], mv[:, 1:2], mybir.ActivationFunctionType.Sqrt, bias=eps_sb[:]
    )
    rstd = pool.tile([P, 1], F32, tag="ln_rstd")
    nc.vector.reciprocal(rstd[:], std[:])
    nc.vector.tensor_scalar(
        z, z,
        scalar1=mv[:, 0:1],
        scalar2=rstd[:],
        op0=mybir.AluOpType.subtract,
        op1=mybir.AluOpType.mult,
    )
    nc.vector.tensor_mul(z, z, g_sb[:])
    nc.vector.tensor_add(z, z, b_sb[:])


def _host_inputs(x, wq, bq, wk, bk, wv, bv, wo, bo, w1, b1, w2, b2,
                 g1, be1, g2, be2):
    """Build the per-core input maps."""
    f = np.float32
    ident = np.eye(P, dtype=f)
    # triangle keep-mask for diagonal blocks: keep iff key p <= query q
    masks = (np.arange(P)[:, None] <= np.arange(P)[None, :]).astype(BF)

    xT = [np.ascontiguousarray(x[b].T).astype(BF) for b in range(B)]

    shared = {
        "wo": np.ascontiguousarray(wo).astype(BF),
        "w1": np.ascontiguousarray(w1).astype(BF),
        "b1": np.ascontiguousarray(b1, dtype=f),
        "w2": np.ascontiguousarray(w2).astype(BF),
        "b2b": np.broadcast_to(b2, (P, D)).astype(f),
        "g1b": np.broadcast_to(g1, (P, D)).astype(f),
        "be1b": np.broadcast_to(be1, (P, D)).astype(f),
        "g2b": np.broadcast_to(g2, (P, D)).astype(f),
        "be2b": np.broadcast_to(be2, (P, D)).astype(f),
        "ident": ident,
        "masks": masks,
    }

    in_maps = []
    for c in range(NCORES):
        b, r = divmod(c, RANKS)
        hs = slice(r * HD, (r + 1) * HD)
        # [q0 q1 | q2 pad | k0 k1 | k2 pad]
        wqkp = np.zeros((D, 4 * P), f)
        wqkp[:, 0:P] = wq[:, hs][:, 0:P]
        wqkp[:, P:P + DK] = wq[:, hs][:, P:HD]
        wqkp[:, 2 * P:3 * P] = wk[:, hs][:, 0:P]
        wqkp[:, 3 * P:3 * P + DK] = wk[:, hs][:, P:HD]
        bqkp = np.zeros(4 * P, f)
        bqkp[0:P] = bq[hs][0:P]
        bqkp[P:P + DK] = bq[hs][P:HD]
        bqkp[2 * P:3 * P] = bk[hs][0:P]
        bqkp[3 * P:3 * P + DK] = bk[hs][P:HD]
        # owned rows: strip r of each 512-chunk, with bo folded in
        xown = np.concatenate(
            [x[b, cc * 512 + r * P: cc * 512 + (r + 1) * P] for cc in range(QC)],
            axis=0,
        ) + bo[None, :]
        selm = np.zeros((P, RANKS), f)
        selm[:, r] = 1.0
        m = {
            "xbT": xT[b],
            "xownbo": np.ascontiguousarray(xown, dtype=f),
            "wqk": wqkp.astype(BF),
            "bqk": bqkp,
            "wvp": np.ascontiguousarray(wv[:, hs]).astype(BF),
            "bvb": np.broadcast_to(bv[hs], (P, HD)).astype(BF),
            "sel": selm,
        }
        m.update(shared)
        in_maps.append({k: np.ascontiguousarray(v) for k, v in m.items()})
    return in_maps


def _get_nc():
    if "nc" not in _CACHE:
        _CACHE["nc"] = _build()
    return _CACHE["nc"]


def run(inputs, **kw):
    """Run on hardware; returns (output, BassKernelResults)."""
    nc = _get_nc()
    in_maps = _host_inputs(**inputs)
    res = run_bass_kernel_spmd(nc, in_maps, core_ids=list(range(NCORES)), **kw)
    out = np.empty((B, S, D), np.float32)
    for core in range(NCORES):
        b, r = divmod(core, RANKS)
        o = res.results[core]["out"]
        for c in range(QC):
            out[b, c * 512 + r * P: c * 512 + (r + 1) * P, :] = o[c * P:(c + 1) * P]
    return out, res


def kernel(**inputs):
    return run(inputs)[0]
